# revision 5
# baseline (speedup 1.0000x reference)
"""Trainium2 Bass kernel for a 3-layer dense GCN (nn_GCN_13846974562486).

Math (reference):
    h1 = relu(adj @ (x  @ W1) + b1)   # [N, 32]
    h2 = relu(adj @ (h1 @ W2) + b2)   # [N, 48]
    h3 = relu(adj @ (h2 @ W3) + b3)   # [N, 64]
    y  = softmax(relu(mean(h3, 0) @ fcW1 + fcb1) @ fcW2 + fcb2)

Distribution: 1D row-shard of adj / output nodes over 8 cores. Each core
holds AT_c = adj[rows_c, :].T  (i.e. adj^T column-block, 128 MiB) and
computes its 2048 output rows per layer:
    O[m, f] = sum_k AT_c[k, m] * z[k, f]
as PE matmuls with the AT tile in the stationary (weight) slot — for fp32
the weight-load path moves 128 elem/cycle vs the streaming path's ~32, so
the PE stays under the DMA roofline (adj is read once per layer: the
memory-bound term, 3 x 128 MiB per core).

Between layers the [N, F] activations z_{l+1} = relu-ed h_l @ W_{l+1} are
AllGathered (2-4 MiB). The mean-pool partial sum [64] is computed on
device per core; the 8 partials and the tiny MLP head are combined on the
host (exact fp32, ~100 kFLOP).
"""

import os
import sys

for _p in ("/opt/trn_rl_repo", "/root/.axon_site/_ro/trn_rl_repo"):
    if os.path.isdir(_p) and _p not in sys.path:
        sys.path.insert(0, _p)

from contextlib import ExitStack

import numpy as np

import concourse.bass as bass
import concourse.mybir as mybir
import concourse.tile as tile
from concourse import bacc
from concourse.bass_utils import run_bass_kernel_spmd
from concourse.masks import make_identity

F32 = mybir.dt.float32

N = 16384           # nodes
NFEAT = 128         # input features
F1, F2, F3 = 32, 48, 64
NCORES = 8
R = N // NCORES     # rows (output nodes) per core = 2048
KT = N // 128       # k-tiles per layer = 128
MT = R // 128       # m-tiles per core = 16
KB = 32             # k-tiles per DMA chunk (2 MiB per dma_start)


def _ts(i, s):
    return slice(i * s, (i + 1) * s)


def _build_nc():
    nc = bacc.Bacc(
        "TRN2", target_bir_lowering=False, debug=False, num_devices=NCORES
    )

    at = nc.dram_tensor("at", [N, R], F32, kind="ExternalInput")
    xt = nc.dram_tensor("xt", [NFEAT, N], F32, kind="ExternalInput")
    w1 = nc.dram_tensor("w1", [NFEAT, F1], F32, kind="ExternalInput")
    w2 = nc.dram_tensor("w2", [F1, F2], F32, kind="ExternalInput")
    w3 = nc.dram_tensor("w3", [F2, F3], F32, kind="ExternalInput")
    b1b = nc.dram_tensor("b1b", [128, F1], F32, kind="ExternalInput")
    b2b = nc.dram_tensor("b2b", [128, F2], F32, kind="ExternalInput")
    b3b = nc.dram_tensor("b3b", [128, F3], F32, kind="ExternalInput")
    out = nc.dram_tensor("out", [1, F3], F32, kind="ExternalOutput")

    at_r = at[:].rearrange("(kt p) m -> p kt m", p=128)  # [128, KT, R]

    with tile.TileContext(nc) as tc, ExitStack() as es:
        const = es.enter_context(tc.tile_pool(name="const", bufs=1))
        atp = es.enter_context(tc.tile_pool(name="atp", bufs=3))
        hp = es.enter_context(tc.tile_pool(name="hp", bufs=2))
        htp = es.enter_context(tc.tile_pool(name="htp", bufs=2))
        zlp = es.enter_context(tc.tile_pool(name="zlp", bufs=1))
        accp = es.enter_context(tc.tile_pool(name="accp", bufs=1))
        dram = es.enter_context(tc.tile_pool(name="dram", bufs=1, space="DRAM"))
        p_acc = es.enter_context(tc.tile_pool(name="p_acc", bufs=2, space="PSUM"))
        p_z = es.enter_context(tc.tile_pool(name="p_z", bufs=2, space="PSUM"))
        p_t = es.enter_context(tc.tile_pool(name="p_t", bufs=2, space="PSUM"))
        p_s = es.enter_context(tc.tile_pool(name="p_s", bufs=2, space="PSUM"))

        w1_sb = const.tile([NFEAT, F1], F32)
        w2_sb = const.tile([F1, F2], F32)
        w3_sb = const.tile([F2, F3], F32)
        b1_sb = const.tile([128, F1], F32)
        b2_sb = const.tile([128, F2], F32)
        b3_sb = const.tile([128, F3], F32)
        ones_sb = const.tile([128, 1], F32)
        ident_sb = const.tile([128, 128], F32)
        acc_sb = accp.tile([1, F3], F32)

        nc.sync.dma_start(w1_sb[:], w1[:])
        nc.sync.dma_start(w2_sb[:], w2[:])
        nc.sync.dma_start(w3_sb[:], w3[:])
        nc.sync.dma_start(b1_sb[:], b1b[:])
        nc.sync.dma_start(b2_sb[:], b2b[:])
        nc.sync.dma_start(b3_sb[:], b3b[:])
        nc.any.memset(ones_sb[:], 1.0)
        make_identity(nc, ident_sb[:])
        nc.vector.memset(acc_sb[:], 0.0)

        # collective bounce buffers
        z2_in = dram.tile([R, F2], F32)
        z2_out = dram.tile([N, F2], F32, addr_space="Shared")
        z3_in = dram.tile([R, F3], F32)
        z3_out = dram.tile([N, F3], F32, addr_space="Shared")

        # Pools released mid-trace must pop in LIFO order: open z3p first
        # (longest-lived), then z2p, then z1p.
        z3_es = ExitStack()
        z3p = z3_es.enter_context(tc.tile_pool(name="z3p", bufs=1))
        z2_es = ExitStack()
        z2p = z2_es.enter_context(tc.tile_pool(name="z2p", bufs=1))
        z1_es = ExitStack()
        z1p = z1_es.enter_context(tc.tile_pool(name="z1p", bufs=1))

        # ---- z1 = x @ W1, computed replicated from xT (chunked) ----
        z1_sb = z1p.tile([128, KT, F1], F32)
        XQ = 4096
        with tc.tile_pool(name="xtp", bufs=2) as xtp:
            for q in range(N // XQ):
                xq_sb = xtp.tile([NFEAT, XQ], F32, tag="xq")
                nc.sync.dma_start(xq_sb[:], xt[:, _ts(q, XQ)])
                for jj in range(XQ // 128):
                    j = q * (XQ // 128) + jj
                    pz = p_z.tile([128, F1], F32, tag="pz")
                    nc.tensor.matmul(
                        pz[:], xq_sb[:, _ts(jj, 128)], w1_sb[:],
                        start=True, stop=True,
                    )
                    nc.vector.tensor_copy(z1_sb[:, j, :], pz[:])

        def layer(z_sb, f_in, w_sb, b_sb, f_out, znext_local, last):
            for mt in range(MT):
                pacc = p_acc.tile([128, f_in], F32, tag="pacc")
                for kc in range(KT // KB):
                    a_sb = atp.tile([128, KB, 128], F32, tag="a")
                    nc.sync.dma_start(
                        a_sb[:], at_r[:, _ts(kc, KB), _ts(mt, 128)]
                    )
                    for kk in range(KB):
                        kt = kc * KB + kk
                        nc.tensor.matmul(
                            pacc[:],
                            a_sb[:, kk, :],
                            z_sb[:, kt, :],
                            start=(kt == 0),
                            stop=(kt == KT - 1),
                        )
                h_sb = hp.tile([128, f_in], F32, tag="h")
                nc.vector.tensor_tensor(
                    h_sb[:], pacc[:], b_sb[:, :f_in], mybir.AluOpType.add
                )
                nc.vector.tensor_scalar_max(h_sb[:], h_sb[:], 0.0)
                if last:
                    ps = p_s.tile([1, f_in], F32, tag="ps")
                    nc.tensor.matmul(
                        ps[:], ones_sb[:], h_sb[:], start=True, stop=True
                    )
                    nc.vector.tensor_tensor(
                        acc_sb[:], acc_sb[:], ps[:], mybir.AluOpType.add
                    )
                else:
                    pt = p_t.tile([f_in, 128], F32, tag="pt")
                    nc.tensor.transpose(pt[:], h_sb[:], ident_sb[:])
                    ht_sb = htp.tile([f_in, 128], F32, tag="ht")
                    nc.vector.tensor_copy(ht_sb[:], pt[:])
                    pz = p_z.tile([128, f_out], F32, tag="pz")
                    nc.tensor.matmul(
                        pz[:], ht_sb[:], w_sb[:], start=True, stop=True
                    )
                    nc.vector.tensor_copy(znext_local[:, mt, :], pz[:])

        def gather(znl_sb, z_in, z_out, znext_sb, g):
            nc.sync.dma_start(
                z_in[:].rearrange("(mt p) g -> p mt g", p=128), znl_sb[:]
            )
            nc.gpsimd.collective_compute(
                "AllGather",
                mybir.AluOpType.bypass,
                replica_groups=[list(range(NCORES))],
                ins=[z_in.opt()],
                outs=[z_out.opt()],
            )
            nc.sync.dma_start(
                znext_sb[:], z_out[:].rearrange("(kt p) g -> p kt g", p=128)
            )

        # ---- layer 1 ----
        z2l_sb = zlp.tile([128, MT, F2], F32, tag="z2l")
        z2_sb = z2p.tile([128, KT, F2], F32)
        layer(z1_sb, F1, w2_sb, b1_sb, F2, z2l_sb, last=False)
        z1_es.close()
        gather(z2l_sb, z2_in, z2_out, z2_sb, F2)

        # ---- layer 2 ----
        z3l_sb = zlp.tile([128, MT, F3], F32, tag="z3l")
        z3_sb = z3p.tile([128, KT, F3], F32)
        layer(z2_sb, F2, w3_sb, b2_sb, F3, z3l_sb, last=False)
        z2_es.close()
        gather(z3l_sb, z3_in, z3_out, z3_sb, F3)

        # ---- layer 3 + mean-pool partial ----
        layer(z3_sb, F3, None, b3_sb, None, None, last=True)
        z3_es.close()

        out_sb = accp.tile([1, F3], F32)
        nc.vector.tensor_copy(out_sb[:], acc_sb[:])
        nc.sync.dma_start(out[:], out_sb[:])

    nc.compile()
    return nc


_NC_CACHE = []


def _get_nc():
    if not _NC_CACHE:
        _NC_CACHE.append(_build_nc())
    return _NC_CACHE[0]


def make_in_maps(x, adj, W1, W2, W3, b1, b2, b3):
    x = np.ascontiguousarray(x, dtype=np.float32)
    xt = np.ascontiguousarray(x.T)
    b1b = np.ascontiguousarray(np.broadcast_to(b1, (128, F1)), dtype=np.float32)
    b2b = np.ascontiguousarray(np.broadcast_to(b2, (128, F2)), dtype=np.float32)
    b3b = np.ascontiguousarray(np.broadcast_to(b3, (128, F3)), dtype=np.float32)
    common = {
        "xt": xt,
        "w1": np.ascontiguousarray(W1, dtype=np.float32),
        "w2": np.ascontiguousarray(W2, dtype=np.float32),
        "w3": np.ascontiguousarray(W3, dtype=np.float32),
        "b1b": b1b,
        "b2b": b2b,
        "b3b": b3b,
    }
    in_maps = []
    for c in range(NCORES):
        at_c = np.ascontiguousarray(adj[c * R : (c + 1) * R, :].T)
        in_maps.append({"at": at_c, **common})
    return in_maps


def head(h3_sum, fcW1, fcb1, fcW2, fcb2):
    y = (h3_sum / np.float32(N)).astype(np.float32)
    y = np.maximum(y @ fcW1 + fcb1, np.float32(0.0))
    logits = y @ fcW2 + fcb2
    e = np.exp(logits - logits.max())
    return (e / e.sum()).astype(np.float32)


def kernel(
    x,
    adj,
    idx_map,  # unused by the reference model
    W1,
    b1,
    W2,
    b2,
    W3,
    b3,
    fcW1,
    fcb1,
    fcW2,
    fcb2,
):
    nc = _get_nc()
    in_maps = make_in_maps(x, adj, W1, W2, W3, b1, b2, b3)
    res = run_bass_kernel_spmd(nc, in_maps, core_ids=list(range(NCORES)))
    h3_sum = np.sum(
        [res.results[c]["out"][0] for c in range(NCORES)], axis=0
    ).astype(np.float32)
    return head(h3_sum, fcW1, fcb1, fcW2, fcb2)
